# revision 30
# baseline (speedup 1.0000x reference)
"""Trainium2 Bass kernel for nn_Attention_39865886442202 (sparse periodic local attention).

Design (v4):
  - Data-parallel over batch B=8 across 8 NeuronCores (one batch element per core).
  - Tokens regrouped by residue (grouped col g = p*16 + t for token n = 128*t + p)
    straight from HBM via DMA access patterns; whole kernel works in grouped space.
  - CHUNK-MAJOR scores: for key chunk cg (128 keys = 8 residues), the attending
    queries form ONE contiguous grouped-column window (208-336 cols), so scores
    are a single matmul per (chunk, head), plus one rank-8 mask matmul
    (key-residue one-hot x per-chunk mask rows).  4 heads packed in PE row strips.
  - exp() batched 4 heads per ScalarE activation (no max-subtraction; scores tiny).
  - AV computed transposed (V stationary [keys, 32voc]) reading the 2-3 exp'd
    chunk windows that overlap each query tile -> attnout^T directly, no output
    transposes.  Softmax denominators via all-ones stationary matmuls; fast
    approximate reciprocal on VectorE.
  - Grp-sequential rounds (heads 0-3 for all chunks, then 4-7) with a lag-2
    software pipeline: round r runs chunk r's scores+exp, then ALL of query
    tile (r-2)'s AV/Z pieces, normalize and final projection -- every
    dependency is >=1 round old so the in-order engine streams overlap.
  - AV / Z / projection psum accumulators live in the spare regions of the
    NEXT round's score PSUM banks (score region uses <=336 of each 512-col
    bank range); two 4-bank score slots cover all of PSUM double-buffered.
"""

import math

import ml_dtypes
import numpy as np

import concourse.bass as bass
import concourse.mybir as mybir
import concourse.tile as tile
from concourse import bacc, bass_utils

DIM = 256
NUM_HEADS = 8
HEAD_DIM = 32
SCALE = HEAD_DIM ** (-0.5)
B = 8
N = 2048
W = 128
T = 16            # token blocks of 128 (and residue tiles of 8)
NEG = -30000.0
FDMAX = 336
AV_OFF = 336      # av region inside psS bank 0 spare
Z_OFF = 512 + 336   # z region inside psS bank 1 spare
PRJ_OFF0 = 1024 + 336  # proj oc 0:128 in bank 2 spare
PRJ_OFF1 = 1536 + 336  # proj oc 128:256 in bank 3 spare

_CACHE = {}
LAST_EXEC_NS = None


def _window(p):
    """Valid key residues [lo, hi) for query residue p (from the torch mask)."""
    if p <= 5:
        return (0, 11)
    if p >= 122:
        return (117, 128)
    return (p - 5, p + 6)


def _blocks(k):
    """Score blocks for query tile k: list of (key chunk cg, rlo, rhi)."""
    if k == 0:
        return [(0, 0, 8), (1, 0, 8)]
    if k == 15:
        return [(14, 0, 8), (15, 0, 8)]
    return [(k - 1, 0, 5), (k, 0, 8), (k + 1, 3, 8)]


def _cg_start(cg):
    return 0 if cg <= 1 else 128 * cg - 80


def _cg_end(cg):
    return N if cg >= 14 else 128 * cg + 208


def _cg_fd(cg):
    return _cg_end(cg) - _cg_start(cg)


def _build_consts():
    bf = ml_dtypes.bfloat16
    # key-residue one-hot, replicated at 4 partition bases
    aone = np.zeros((128, 128), dtype=np.float32)
    for g in range(4):
        for j in range(8):
            aone[32 * g + j, 16 * j:16 * (j + 1)] = 1.0
    # chunk-major mask values: per chunk cg, per query column of its window
    koffs = []
    o = 0
    for cg in range(T):
        koffs.append(o)
        o += _cg_fd(cg)
    maskb = np.zeros((128, o), dtype=np.float32)
    for cg in range(T):
        s = _cg_start(cg)
        for c in range(_cg_fd(cg)):
            qg = s + c
            k, r = qg // 128, (qg % 128) // 16
            lo, hi = _window(8 * k + r)
            for j in range(8):
                val = 0.0 if lo <= 8 * cg + j < hi else NEG
                for g in range(4):
                    maskb[32 * g + j, koffs[cg] + c] = val
    ident = np.eye(128, dtype=np.float32)
    return aone.astype(bf), maskb.astype(bf), ident.astype(bf), koffs, o


def _build_program():
    _, _, _, koffs, mbw = _build_consts()
    nc = bacc.Bacc(None, target_bir_lowering=False)
    f32 = mybir.dt.float32
    bf16 = mybir.dt.bfloat16

    x_in = nc.declare_dram_parameter("x", [N, DIM], f32, isOutput=False)
    wqkv_in = nc.declare_dram_parameter("wqkv", [DIM, 3 * DIM], f32, isOutput=False)
    wproj_in = nc.declare_dram_parameter("wproj", [DIM, DIM], f32, isOutput=False)
    bproj_in = nc.declare_dram_parameter("bproj", [DIM], f32, isOutput=False)
    aone_in = nc.declare_dram_parameter("aone", [128, 128], bf16, isOutput=False)
    maskb_in = nc.declare_dram_parameter("maskb", [128, mbw], bf16, isOutput=False)
    ident_in = nc.declare_dram_parameter("ident", [128, 128], bf16, isOutput=False)
    out_ext = nc.declare_dram_parameter("out", [N, DIM], f32, isOutput=True)

    # grouped view of x / out: token n = 128*t + 8*pm + pl -> chunk pm, row pl*16+t
    xg = x_in.rearrange("(t pm pl) d -> pl t pm d", pm=16, pl=8)
    outg = out_ext.rearrange("(t pm pl) d -> pl t pm d", pm=16, pl=8)

    with tile.TileContext(nc) as tc:
        with (
            tc.tile_pool(name="singles", bufs=1) as singles,
            tc.tile_pool(name="sbw", bufs=4) as sbw,
            tc.tile_pool(name="sbz", bufs=4) as sbz,
            tc.tile_pool(name="sbo", bufs=4) as sbo,
        ):
            # ---- x load (grouped rows) first, on both HWDGE queues ----
            xraw = singles.tile([128, 16 * DIM], f32)
            xraw3 = xraw.rearrange("q (m d) -> q m d", m=16)
            xbf = singles.tile([128, 16 * DIM], bf16)
            xbf3 = xbf.rearrange("q (m d) -> q m d", m=16)
            dmaq = [nc.sync, nc.scalar]
            for m in range(16):
                dmaq[m % 2].dma_start(out=xraw3[:, m, :], in_=xg[:, :, m, :])

            # ---- constants / weights ----
            ident_sb = singles.tile([128, 128], bf16)
            nc.sync.dma_start(out=ident_sb, in_=ident_in[:, :])
            aone_sb = singles.tile([128, 128], bf16)
            nc.sync.dma_start(out=aone_sb, in_=aone_in[:, :])
            maskb_sb = singles.tile([128, mbw], bf16)
            nc.sync.dma_start(out=maskb_sb, in_=maskb_in[:, :])

            wqkv_sb = []
            for dc in range(2):
                t_ = singles.tile([128, 3 * DIM], bf16, tag=f"wqkv{dc}")
                nc.gpsimd.dma_start(out=t_, in_=wqkv_in[128 * dc:128 * (dc + 1), :])
                wqkv_sb.append(t_)
            wproj_sb = []
            for fc in range(2):
                t_ = singles.tile([128, DIM], bf16, tag=f"wproj{fc}")
                nc.gpsimd.dma_start(out=t_, in_=wproj_in[128 * fc:128 * (fc + 1), :])
                wproj_sb.append(t_)
            biasrow = singles.tile([1, DIM], bf16)
            bp = bproj_in[:]
            nc.gpsimd.dma_start(
                out=biasrow,
                in_=bass.AP(tensor=bp.tensor, offset=bp.offset, ap=[[0, 1], [1, DIM]]),
            )
            ones1 = singles.tile([1, 128], bf16)
            nc.gpsimd.memset(ones1, 1.0)
            onesT = singles.tile([128, 32], bf16)
            nc.gpsimd.memset(onesT, 1.0)

            # ---- persistent SBUF tensors ----
            xTg = [singles.tile([128, N], bf16, name=f"xTg{dc}", tag=f"xTg{dc}")
                   for dc in range(2)]
            qT = [singles.tile([128, N], bf16, name=f"qT{g}", tag=f"qT{g}")
                  for g in range(2)]
            kT = [singles.tile([128, N], bf16, name=f"kT{g}", tag=f"kT{g}")
                  for g in range(2)]
            vsb = singles.tile([128, 16 * DIM], bf16)
            aoT = [singles.tile([128, N], bf16, name=f"aoT{g}", tag=f"aoT{g}")
                   for g in range(2)]

            # ---- phase A: transpose + QKV projections ----
            with (
                tc.tile_pool(name="pstp", bufs=3, space="PSUM") as pstp,
                tc.tile_pool(name="pspj", bufs=2, space="PSUM") as pspj,
            ):
                for m in range(16):
                    nc.vector.tensor_copy(xbf3[:, m, :], xraw3[:, m, :])
                for dc in range(2):
                    for mg in range(4):
                        tp = pstp.tile([128, 512], bf16, tag="tp")
                        for mi in range(4):
                            m = 4 * mg + mi
                            nc.tensor.transpose(
                                tp[:, 128 * mi:128 * (mi + 1)],
                                xbf3[:, m, 128 * dc:128 * (dc + 1)],
                                ident_sb,
                            )
                        nc.vector.tensor_copy(
                            xTg[dc][:, 512 * mg:512 * (mg + 1)], tp
                        )

                # Q/K projections: oc4 0,1 -> Q head groups; 2,3 -> K.
                # grp0 tensors and V first so attention can start earlier.
                def qk_proj(oc4):
                    dest = (qT[0], qT[1], kT[0], kT[1])[oc4]
                    for half in range(2):
                        ps = pspj.tile([128, 1024], f32, tag="pj",
                                       name=f"pj{oc4}_{half}")
                        for nf in range(2):
                            for dc in range(2):
                                nc.tensor.matmul(
                                    ps[:, 512 * nf:512 * (nf + 1)],
                                    lhsT=wqkv_sb[dc][:, 128 * oc4:128 * (oc4 + 1)],
                                    rhs=xTg[dc][:, 1024 * half + 512 * nf:
                                                1024 * half + 512 * (nf + 1)],
                                    start=(dc == 0), stop=(dc == 1),
                                )
                        if half == 0:
                            nc.vector.tensor_copy(
                                dest[:, 1024 * half:1024 * (half + 1)], ps)
                        else:
                            nc.scalar.copy(
                                dest[:, 1024 * half:1024 * (half + 1)], ps)

                def v_proj():
                    for mq in range(4):
                        ps = pspj.tile([128, 1024], f32, tag="pj",
                                       name=f"pjv{mq}")
                        for mi in range(4):
                            m = 4 * mq + mi
                            for dc in range(2):
                                nc.tensor.matmul(
                                    ps[:, 256 * mi:256 * (mi + 1)],
                                    lhsT=xTg[dc][:, 128 * m:128 * (m + 1)],
                                    rhs=wqkv_sb[dc][:, 2 * DIM:3 * DIM],
                                    start=(dc == 0), stop=(dc == 1),
                                )
                        if mq % 2 == 0:
                            nc.vector.tensor_copy(
                                vsb[:, 1024 * mq:1024 * (mq + 1)], ps)
                        else:
                            nc.scalar.copy(
                                vsb[:, 1024 * mq:1024 * (mq + 1)], ps)

                qk_proj(0)
                qk_proj(2)
                v_proj()
                qk_proj(1)
                qk_proj(3)

            # ---- phase B: attention (+ fused final projection) ----
            with tc.tile_pool(name="psb", bufs=2, space="PSUM") as psb:
                slots = {}
                ptils = {}

                def emit_head(cg, grp):
                    fd = _cg_fd(cg)
                    s = _cg_start(cg)
                    qTg, kTg = qT[grp], kT[grp]
                    psS = psb.tile([128, 2048], f32, tag="psS",
                                   name=f"psS{cg}_{grp}")
                    ps3 = psS.rearrange("p (h c) -> p h c", h=4)
                    for hh in range(4):
                        base = 32 * hh
                        nc.tensor.matmul(
                            ps3[:, hh, 0:fd],
                            lhsT=kTg[base:base + 32, 128 * cg:128 * (cg + 1)],
                            rhs=qTg[base:base + 32, s:s + fd],
                            start=True, stop=False,
                            tile_position=(base, 0),
                        )
                        nc.tensor.matmul(
                            ps3[:, hh, 0:fd],
                            lhsT=aone_sb[base:base + 8, :],
                            rhs=maskb_sb[base:base + 8, koffs[cg]:koffs[cg] + fd],
                            start=False, stop=True,
                            tile_position=(base, 0),
                        )
                    ptil = sbw.tile([128, 4 * FDMAX], bf16, tag="ptil",
                                    name=f"ptil{cg}_{grp}")
                    pt3 = ptil.rearrange("p (h c) -> p h c", h=4)
                    nc.scalar.activation(
                        pt3[:, :, 0:fd], ps3[:, :, 0:fd],
                        mybir.ActivationFunctionType.Exp,
                    )
                    slots[(cg, grp)] = psS
                    ptils[(cg, grp)] = pt3

                def slot_regions(k, grp):
                    """(psum tile, av, z offsets) for tile k's accumulators."""
                    if k < T - 1:
                        return slots[(k + 1, grp)], AV_OFF, Z_OFF
                    return slots[(T - 1, grp)], 208, 512 + 208

                def emit_piece(k, cg, grp):
                    pt3 = ptils[(cg, grp)]
                    _, rlo, rhi = [b for b in _blocks(k) if b[0] == cg][0]
                    nq = 16 * (rhi - rlo)
                    clo = 128 * k + 16 * rlo - _cg_start(cg)
                    psS, av_o, z_o = slot_regions(k, grp)
                    last = (cg == _blocks(k)[-1][0])
                    for hh in range(4):
                        h = 4 * grp + hh
                        nc.tensor.matmul(
                            psS[32 * hh:32 * (hh + 1),
                                av_o + 16 * rlo:av_o + 16 * rlo + nq],
                            lhsT=vsb[:, 256 * cg + 32 * h:
                                     256 * cg + 32 * (h + 1)],
                            rhs=pt3[:, hh, clo:clo + nq],
                            start=False, stop=last,
                            tile_position=(0, 32 * hh),
                            skip_group_check=True,
                        )
                        nc.tensor.matmul(
                            psS[32 * hh:32 * (hh + 1),
                                z_o + 16 * rlo:z_o + 16 * rlo + nq],
                            lhsT=onesT[:, :],
                            rhs=pt3[:, hh, clo:clo + nq],
                            start=False, stop=last,
                            tile_position=(0, 32 * hh),
                            skip_group_check=True,
                        )


                def emit_norm(k, grp):
                    psS, av_o, z_o = slot_regions(k, grp)
                    zrec = sbz.tile([128, 128], f32, tag="zrec",
                                    name=f"zrec{k}_{grp}")
                    nc.vector.reciprocal_approx_fast(
                        out=zrec, in_=psS[:, z_o:z_o + 128])
                    nc.vector.tensor_mul(
                        aoT[grp][:, 128 * k:128 * (k + 1)],
                        psS[:, av_o:av_o + 128],
                        zrec,
                    )

                def emit_proj(j, psS, p0_o, p1_o):
                    for och in range(2):
                        poff = (p0_o, p1_o)[och]
                        for fc in range(2):
                            nc.tensor.matmul(
                                psS[:, poff:poff + 128],
                                lhsT=aoT[fc][:, 128 * j:128 * (j + 1)],
                                rhs=wproj_sb[fc][:, 128 * och:128 * (och + 1)],
                                start=(fc == 0), stop=False,
                                skip_group_check=True,
                            )
                        nc.tensor.matmul(
                            psS[:, poff:poff + 128],
                            lhsT=ones1[:, :],
                            rhs=biasrow[:, 128 * och:128 * (och + 1)],
                            start=False, stop=True,
                            skip_group_check=True,
                        )
                        osb = sbo.tile([128, 128], f32, tag="osb",
                                       name=f"osb{j}_{och}")
                        nc.vector.tensor_copy(osb, psS[:, poff:poff + 128])
                        nc.sync.dma_start(
                            out=outg[:, :, j, 128 * och:128 * (och + 1)],
                            in_=osb,
                        )

                # All of tile k's pieces/norm/proj run at round k+2, AFTER
                # round k+2's head: every dependency (ptil exps, slot bank
                # WARs, norms) is then at least ~1 round old, so the per-engine
                # in-order streams never stall and the pipeline self-paces.
                for grp in range(2):
                    for r in range(T + 2):
                        if r < T:
                            emit_head(r, grp)
                        k = r - 2
                        if 0 <= k < T:
                            for cg, _, _ in _blocks(k):
                                emit_piece(k, cg, grp)
                            emit_norm(k, grp)
                            if grp == 1 and k >= 0:
                                if k <= T - 3:
                                    emit_proj(k, slots[(k + 2, grp)],
                                              PRJ_OFF0, PRJ_OFF1)
                                elif k == T - 2:
                                    emit_proj(k, slots[(T - 1, grp)],
                                              1024 + 208, 1536 + 208)
                                else:
                                    emit_proj(k, slots[(T - 2, grp)],
                                              PRJ_OFF0, PRJ_OFF1)
                    ptils.clear()
                    slots.clear()
    nc.finalize()
    return nc


def kernel(x, w, Wqkv, Wproj, bproj, **kw):
    global LAST_EXEC_NS
    assert int(w) == W
    x = np.asarray(x, dtype=np.float32)
    Wqkv = np.asarray(Wqkv, dtype=np.float32).copy()
    Wproj = np.asarray(Wproj, dtype=np.float32)
    bproj = np.asarray(bproj, dtype=np.float32)
    Wqkv[:, :DIM] = Wqkv[:, :DIM] * SCALE  # fold attention scale into Wq

    if "prog" not in _CACHE:
        _CACHE["prog"] = _build_program()
        _CACHE["consts"] = _build_consts()
    nc = _CACHE["prog"]
    aone, maskb, ident, _, _ = _CACHE["consts"]

    core_ids = list(range(B))
    in_maps = []
    for b in range(B):
        in_maps.append({
            "x": np.ascontiguousarray(x[b]),
            "wqkv": Wqkv,
            "wproj": Wproj,
            "bproj": bproj,
            "aone": aone,
            "maskb": maskb,
            "ident": ident,
        })
    res = bass_utils.run_bass_kernel_spmd(nc, in_maps, core_ids)
    globals()["LAST_RES"] = res
    LAST_EXEC_NS = res.exec_time_ns
    out = np.stack([res.results[b]["out"] for b in range(B)], axis=0)
    return out.astype(np.float32)
